# revision 27
# baseline (speedup 1.0000x reference)
"""Trainium2 Bass kernel for nn_CrossLayer (DCN-style cross stack).

Reference semantics (B=16384, D=1024, L=8):
    out_0 = x
    s_i = einsum('bd,d->b', out_i, W[i])
    out_{i+1} = x * s_i[:, None] + b[i] + x

Algebraic collapse: out_{i+1} = x * rho_{i+1} + b[i] with
    rho_1 = u_0 + 1,   rho_{l+1} = rho_l * u_l + c_l
    u_l[r] = <x[r, :], W[l]>          (U = x @ W.T, [B, L])
    c_l = <b[l-1], W[l]> + 1          (weights-only scalars)
    out = x * rho_8[:, None] + b[L-1]

Device work: U = x @ W.T via PE transposes + W-stationary matmuls, all in
float32r (1-pass PE datapath: transpose 1.5 cyc/row, matmul 1 cyc/row at
>=256 moving cols, vs 2/4 for plain fp32), an 8-step per-row scan on DVE
(initial=1, c_0=1 folds the +1 into the scan), one fused scale+bias pass.

Memory layout: 256-row blocks where partition p holds DRAM rows 2p/2p+1
of the block -> every x/y DMA descriptor is 8KB contiguous. The row
permutation is never undone: transposes, scan, fuse, and the output DMA
all use the same (p, slot) mapping. Fully per-block pipeline (no group
barriers): each block's U matmul (ap=256) depends only on its own
transposes, so outputs start flowing as soon as block 0 is fused and
input/output streams overlap for the whole run. Identity is built
on-chip (no slow 512B-descriptor DMA). x read once, out written once ->
memory-roofline bound.

Sharding: data-parallel over batch; 8 cores x 2048 rows. Tiny (L, D)
weights replicated.
"""

import numpy as np

import concourse.bacc as bacc
import concourse.tile as tile
from concourse import mybir
from concourse.bass_utils import run_bass_kernel_spmd
from concourse.masks import make_identity

N_CORES = 8
B, D, L = 16384, 1024, 8
RPC = B // N_CORES          # rows per core (2048)
NB = RPC // 256             # 256-row blocks per core (8)
NCH = D // 128              # 128-wide d chunks (8)
N_WARM = 4                  # bf16 warmup matmuls to hold PE p-state up

LAST_RESULTS = None


def _build(cvals):
    """Trace + compile the per-core program. cvals = [c_1..c_{L-1}]."""
    nc = bacc.Bacc("TRN2", target_bir_lowering=False, debug=False)
    f32 = mybir.dt.float32
    f32r = mybir.dt.float32r
    bf16 = mybir.dt.bfloat16

    # x/wt declared f32r (byte-identical to the f32 numpy payload) so the
    # sync engine can DMA them straight into f32r tiles (no cast) and the
    # BIR fp32r-producer check is satisfied.
    x_d = nc.dram_tensor("x", [RPC, D], f32r, kind="ExternalInput")
    wt_d = nc.dram_tensor("wt", [128, NCH * L], f32r, kind="ExternalInput")
    b7_d = nc.dram_tensor("b7r", [128, D], f32, kind="ExternalInput")
    y_d = nc.dram_tensor("y", [RPC, D], f32, kind="ExternalOutput")

    # block views: partition p <-> rows 2p, 2p+1 of the block (8KB descr.)
    x_blk = x_d.ap().rearrange("(t p r) d -> t p (r d)", p=128, r=2)
    y_blk = y_d.ap().rearrange("(t p r) d -> t p (r d)", p=128, r=2)

    with tile.TileContext(nc) as tc:
        with (
            tc.tile_pool(name="const", bufs=1) as cpool,
            tc.tile_pool(name="xp", bufs=6) as xpool,
            tc.tile_pool(name="xtp", bufs=3) as xtpool,
            tc.tile_pool(name="yp", bufs=4) as ypool,
            tc.tile_pool(name="small", bufs=6) as spool,
            tc.tile_pool(name="pst", bufs=4, space="PSUM") as pst,
            tc.tile_pool(name="psu", bufs=2, space="PSUM") as psu,
            tc.tile_pool(name="psr", bufs=2, space="PSUM") as psr,
        ):
            # --- all x input DMAs issued up front (xpool bufs bound the
            # number actually in flight; SP runs ahead of compute) ---
            xbs = []
            for i in range(NB):
                xb = xpool.tile([128, 2 * D], f32r, tag="xb")
                nc.sync.dma_start(out=xb[:], in_=x_blk[i])
                xbs.append(xb)

            # --- warmup: dense bf16 matmuls during initial DMA window ---
            dummy = cpool.tile([128, 512], bf16)
            nc.gpsimd.memset(dummy[:], 0.0)
            for i in range(N_WARM):
                pw = psr.tile([128, 512], f32, tag="psr")
                nc.tensor.matmul(pw[:], dummy[:, 0:128], dummy[:], start=True, stop=True)

            # --- constants (no big/slow const DMAs on the critical path) ---
            idf = cpool.tile([128, 128], f32)
            make_identity(nc, idf[:])
            ident = cpool.tile([128, 128], f32r)
            nc.scalar.copy(ident[:], idf[:])   # rounds to f32r for the PE
            wt_sb = cpool.tile([128, NCH, L], f32r)
            nc.sync.dma_start(out=wt_sb[:], in_=wt_d.ap().rearrange("p (c l) -> p c l", l=L))
            b7_sb = cpool.tile([128, D], f32)
            nc.sync.dma_start(out=b7_sb[:], in_=b7_d[:, :])
            # scan constants: cc[:, 0] = 1 (folds the +1 of rho_1), cc[:, l] = c_l
            cc_sb = cpool.tile([128, L], f32)
            nc.gpsimd.memset(cc_sb[:, 0:1], 1.0)
            for l in range(1, L):
                nc.gpsimd.memset(cc_sb[:, l : l + 1], cvals[l - 1])
            ones = cpool.tile([128, 1], f32)
            nc.gpsimd.memset(ones[:], 1.0)

            for i in range(NB):
                xb = xbs[i]
                # [p, slot, chunk, 128] and [p, slot, 1024] views
                xb_c = xb[:].rearrange("p (r c d) -> p r c d", r=2, c=NCH)
                xb_f = xb[:].rearrange("p (r d) -> p r d", r=2)

                # transpose chunks -> xT [128d, c, 256]; col = s*128 + p
                xT = xtpool.tile([128, NCH, 256], f32r, tag="xT")
                for s in range(2):
                    off = 128 * s
                    h = NCH // 2
                    pa = pst.tile([128, h, 128], f32, tag="pst")
                    for c in range(h):
                        nc.tensor.transpose(
                            pa[:, c, :].bitcast(f32r), xb_c[:, s, c, :], ident[:]
                        )
                    nc.scalar.copy(xT[:, 0:h, off : off + 128], pa[:].bitcast(f32r))
                    pb = pst.tile([128, h, 128], f32, tag="pst")
                    for c in range(h):
                        nc.tensor.transpose(
                            pb[:, c, :].bitcast(f32r), xb_c[:, s, h + c, :], ident[:]
                        )
                    nc.scalar.copy(xT[:, h:NCH, off : off + 128], pb[:].bitcast(f32r))

                # U^T for the block: [L, 256] = sum_c WT_c.T @ xT_c
                ps_u = psu.tile([L, 256], f32, tag="psu")
                for c in range(NCH):
                    nc.tensor.matmul(
                        ps_u[:], wt_sb[:, c, :], xT[:, c, :],
                        start=(c == 0), stop=(c == NCH - 1),
                    )
                ut = spool.tile([L, 256], f32r, tag="ut")
                nc.scalar.copy(ut[:], ps_u[:])

                yt = ypool.tile([128, 2, D], f32, tag="yt")
                for s in range(2):
                    off = 128 * s
                    # U slot back to row-partition orientation: [128, L]
                    pr = psr.tile([128, L], f32, tag="psr")
                    nc.tensor.transpose(
                        pr[:].bitcast(f32r), ut[:, off : off + 128], ident[0:L, 0:L]
                    )
                    # rho chain: rho_{l+1} = rho_l*u_l + c_l, rho_0 = c_0 = 1
                    scano = spool.tile([128, L], f32, tag="scan")
                    nc.vector.tensor_tensor_scan(
                        scano[:], pr[:], cc_sb[:], ones[:, 0:1],
                        mybir.AluOpType.mult, mybir.AluOpType.add,
                    )
                    # out = x * rho + b7
                    nc.vector.scalar_tensor_tensor(
                        yt[:, s, :], xb_f[:, s, :].bitcast(f32),
                        scano[:, L - 1 : L], b7_sb[:],
                        mybir.AluOpType.mult, mybir.AluOpType.add,
                    )
                nc.gpsimd.dma_start(out=y_blk[i], in_=yt[:])

    nc.compile()
    return nc


def kernel(x, W, b):
    global LAST_RESULTS
    x = np.ascontiguousarray(np.asarray(x), dtype=np.float32)
    W = np.ascontiguousarray(np.asarray(W), dtype=np.float32)
    b = np.ascontiguousarray(np.asarray(b), dtype=np.float32)
    assert x.shape == (B, D) and W.shape == (L, D) and b.shape == (L, D)

    cvals = [float(np.dot(b[l - 1].astype(np.float64), W[l].astype(np.float64)) + 1.0)
             for l in range(1, L)]
    wt = W.T.reshape(NCH, 128, L).transpose(1, 0, 2).reshape(128, NCH * L)
    wt = np.ascontiguousarray(wt, dtype=np.float32)
    b7r = np.ascontiguousarray(np.broadcast_to(b[L - 1], (128, D)), dtype=np.float32)

    nc = _build(cvals)

    shards = [x[i * RPC : (i + 1) * RPC] for i in range(N_CORES)]
    in_maps = [{"x": s, "wt": wt, "b7r": b7r} for s in shards]
    res = run_bass_kernel_spmd(nc, in_maps, core_ids=list(range(N_CORES)))
    LAST_RESULTS = res
    out = np.concatenate([res.results[i]["y"] for i in range(N_CORES)], axis=0)
    return out.astype(np.float32)


# revision 29
# speedup vs baseline: 1.0723x; 1.0723x over previous
"""Trainium2 Bass kernel for nn_CrossLayer (DCN-style cross stack).

Reference semantics (B=16384, D=1024, L=8):
    out_0 = x
    s_i = einsum('bd,d->b', out_i, W[i])
    out_{i+1} = x * s_i[:, None] + b[i] + x

Algebraic collapse: out_{i+1} = x * rho_{i+1} + b[i] with
    rho_1 = u_0 + 1,   rho_{l+1} = rho_l * u_l + c_l
    u_l[r] = <x[r, :], W[l]>          (U = x @ W.T, [B, L])
    c_l = <b[l-1], W[l]> + 1          (weights-only scalars)
    out = x * rho_8[:, None] + b[L-1]

Device work: U = x @ W.T via PE transposes + W-stationary matmuls, all in
float32r (1-pass PE datapath: transpose 1.5 cyc/row, matmul 1 cyc/row at
>=256 moving cols, vs 2/4 for plain fp32), an 8-step per-row scan on DVE
(initial=1, c_0=1 folds the +1 into the scan), one fused scale+bias pass
per 128-row slot (slot 0 on DVE, slot 1 on gpsimd to halve the pacing).

Memory layout: 256-row blocks where partition p holds DRAM rows 2p/2p+1
of the block -> every x/y DMA descriptor is 8KB contiguous. The row
permutation is never undone: transposes, scan, fuse, and the output DMA
all use the same (p, slot) mapping. Fully per-block pipeline: constants
are loaded first, all 8 input-block DMAs are issued up front (bufs=8, so
the sync queue never stalls on buffer recycling), and each block's U
matmul (ap=256) depends only on its own transposes, so outputs start
flowing ~15us in and input/output streams overlap for the whole run.
x read once, out written once -> memory-roofline bound.

Sharding: data-parallel over batch; 8 cores x 2048 rows. Tiny (L, D)
weights replicated.
"""

import numpy as np

import concourse.bacc as bacc
import concourse.tile as tile
from concourse import mybir
from concourse.bass_utils import run_bass_kernel_spmd
from concourse.masks import make_identity

N_CORES = 8
B, D, L = 16384, 1024, 8
RPC = B // N_CORES          # rows per core (2048)
NB = RPC // 256             # 256-row blocks per core (8)
NCH = D // 128              # 128-wide d chunks (8)
N_WARM = 4                  # bf16 warmup matmuls to hold PE p-state up

LAST_RESULTS = None


def _build(cvals):
    """Trace + compile the per-core program. cvals = [c_1..c_{L-1}]."""
    nc = bacc.Bacc("TRN2", target_bir_lowering=False, debug=False)
    f32 = mybir.dt.float32
    f32r = mybir.dt.float32r
    bf16 = mybir.dt.bfloat16

    # x/wt declared f32r (byte-identical to the f32 numpy payload) so the
    # sync engine can DMA them straight into f32r tiles (no cast) and the
    # BIR fp32r-producer check is satisfied.
    x_d = nc.dram_tensor("x", [RPC, D], f32r, kind="ExternalInput")
    wt_d = nc.dram_tensor("wt", [128, NCH * L], f32r, kind="ExternalInput")
    b7_d = nc.dram_tensor("b7r", [128, D], f32, kind="ExternalInput")
    y_d = nc.dram_tensor("y", [RPC, D], f32, kind="ExternalOutput")

    # block views: partition p <-> rows 2p, 2p+1 of the block (8KB descr.)
    x_blk = x_d.ap().rearrange("(t p r) d -> t p (r d)", p=128, r=2)
    y_blk = y_d.ap().rearrange("(t p r) d -> t p (r d)", p=128, r=2)

    with tile.TileContext(nc) as tc:
        with (
            tc.tile_pool(name="const", bufs=1) as cpool,
            tc.tile_pool(name="xp", bufs=8) as xpool,
            tc.tile_pool(name="xtp", bufs=3) as xtpool,
            tc.tile_pool(name="yp", bufs=4) as ypool,
            tc.tile_pool(name="small", bufs=6) as spool,
            tc.tile_pool(name="pst", bufs=2, space="PSUM") as pst,
            tc.tile_pool(name="psu", bufs=2, space="PSUM") as psu,
            tc.tile_pool(name="psr", bufs=2, space="PSUM") as psr,
        ):
            # --- small constants first so nothing blocks the fuse path ---
            b7_sb = cpool.tile([128, D], f32)
            nc.sync.dma_start(out=b7_sb[:], in_=b7_d[:, :])
            wt_sb = cpool.tile([128, NCH, L], f32r)
            nc.sync.dma_start(out=wt_sb[:], in_=wt_d.ap().rearrange("p (c l) -> p c l", l=L))

            # --- all x input DMAs issued up front; bufs=8 keeps the sync
            # queue from ever stalling on buffer recycling ---
            xbs = []
            for i in range(NB):
                xb = xpool.tile([128, 2 * D], f32r, tag="xb")
                nc.sync.dma_start(out=xb[:], in_=x_blk[i])
                xbs.append(xb)

            # --- warmup: dense bf16 matmuls during initial DMA window ---
            dummy = cpool.tile([128, 512], bf16)
            nc.gpsimd.memset(dummy[:], 0.0)
            for i in range(N_WARM):
                pw = psr.tile([128, 512], f32, tag="psr")
                nc.tensor.matmul(pw[:], dummy[:, 0:128], dummy[:], start=True, stop=True)

            # --- identity built on-chip (fp32), rounded to f32r ---
            idf = cpool.tile([128, 128], f32)
            make_identity(nc, idf[:])
            ident = cpool.tile([128, 128], f32r)
            nc.scalar.copy(ident[:], idf[:])
            # scan constants: cc[:, 0] = 1 (folds the +1 of rho_1), cc[:, l] = c_l
            cc_sb = cpool.tile([128, L], f32)
            nc.gpsimd.memset(cc_sb[:, 0:1], 1.0)
            for l in range(1, L):
                nc.gpsimd.memset(cc_sb[:, l : l + 1], cvals[l - 1])
            ones = cpool.tile([128, 1], f32)
            nc.gpsimd.memset(ones[:], 1.0)

            for i in range(NB):
                xb = xbs[i]
                # [p, slot, chunk, 128] and [p, slot, 1024] views
                xb_c = xb[:].rearrange("p (r c d) -> p r c d", r=2, c=NCH)
                xb_f = xb[:].rearrange("p (r d) -> p r d", r=2)

                # transpose chunks -> xT [128d, c, 256]; col = s*128 + p
                xT = xtpool.tile([128, NCH, 256], f32r, tag="xT")
                for s in range(2):
                    off = 128 * s
                    pt = pst.tile([128, NCH, 128], f32, tag="pst")
                    for c in range(NCH):
                        nc.tensor.transpose(
                            pt[:, c, :].bitcast(f32r), xb_c[:, s, c, :], ident[:]
                        )
                    nc.scalar.copy(xT[:, :, off : off + 128], pt[:].bitcast(f32r))

                # U^T for the block: [L, 256] = sum_c WT_c.T @ xT_c
                ps_u = psu.tile([L, 256], f32, tag="psu")
                for c in range(NCH):
                    nc.tensor.matmul(
                        ps_u[:], wt_sb[:, c, :], xT[:, c, :],
                        start=(c == 0), stop=(c == NCH - 1),
                    )
                ut = spool.tile([L, 256], f32r, tag="ut")
                nc.scalar.copy(ut[:], ps_u[:])

                yt = ypool.tile([128, 2, D], f32, tag="yt")
                for s in range(2):
                    off = 128 * s
                    # U slot back to row-partition orientation: [128, L]
                    pr = psr.tile([128, L], f32, tag="psr")
                    nc.tensor.transpose(
                        pr[:].bitcast(f32r), ut[:, off : off + 128], ident[0:L, 0:L]
                    )
                    # rho chain: rho_{l+1} = rho_l*u_l + c_l, rho_0 = c_0 = 1
                    scano = spool.tile([128, L], f32, tag="scan")
                    nc.vector.tensor_tensor_scan(
                        scano[:], pr[:], cc_sb[:], ones[:, 0:1],
                        mybir.AluOpType.mult, mybir.AluOpType.add,
                    )
                    # out = x * rho + b7
                    nc.vector.scalar_tensor_tensor(
                        yt[:, s, :], xb_f[:, s, :].bitcast(f32),
                        scano[:, L - 1 : L], b7_sb[:],
                        mybir.AluOpType.mult, mybir.AluOpType.add,
                    )
                nc.gpsimd.dma_start(out=y_blk[i], in_=yt[:])

    nc.compile()
    return nc


def kernel(x, W, b):
    global LAST_RESULTS
    x = np.ascontiguousarray(np.asarray(x), dtype=np.float32)
    W = np.ascontiguousarray(np.asarray(W), dtype=np.float32)
    b = np.ascontiguousarray(np.asarray(b), dtype=np.float32)
    assert x.shape == (B, D) and W.shape == (L, D) and b.shape == (L, D)

    cvals = [float(np.dot(b[l - 1].astype(np.float64), W[l].astype(np.float64)) + 1.0)
             for l in range(1, L)]
    wt = W.T.reshape(NCH, 128, L).transpose(1, 0, 2).reshape(128, NCH * L)
    wt = np.ascontiguousarray(wt, dtype=np.float32)
    b7r = np.ascontiguousarray(np.broadcast_to(b[L - 1], (128, D)), dtype=np.float32)

    nc = _build(cvals)

    shards = [x[i * RPC : (i + 1) * RPC] for i in range(N_CORES)]
    in_maps = [{"x": s, "wt": wt, "b7r": b7r} for s in shards]
    res = run_bass_kernel_spmd(nc, in_maps, core_ids=list(range(N_CORES)))
    LAST_RESULTS = res
    out = np.concatenate([res.results[i]["y"] for i in range(N_CORES)], axis=0)
    return out.astype(np.float32)
